# revision 1
# baseline (speedup 1.0000x reference)
"""Trainium2 Bass kernel for nn_DeepLinear (784->10 linear + BN, 62x(10->10 linear + BN), 10->10 linear).

Math: BN output has exact per-column batch mean beta, so every layer past the first
acts linearly on the *centered* activations. The whole net collapses to:
    h  = x @ W0.T                      (heavy, on device, data-parallel over batch)
    mu = mean(h), C = cov(h)           (global batch moments; partial moments per core,
                                        combined on host = the sync-BN all-reduce)
    T, r = 62-layer chain of 10x10 covariance algebra (tiny, host, float64)
    out = (h - mu) @ T + r             (light, on device)

The big matmul runs as an exact fp16 hi/lo split: x = xh + xl, W0 = Wh + Wl in fp16,
accumulating all four cross terms in fp32 PSUM -> bit-accuracy ~fp32 at 1 cycle/row.
"""

import numpy as np

EPS = 1e-5
B = 65536
D = 784
NCORES = 8
BC = B // NCORES          # 8192 rows per core
KP = 112                  # contraction chunk partitions (7 * 112 = 784)
KC = 7                    # contraction chunks
CB = 512                  # batch columns per DMA chunk
NT = 512                  # batch columns per matmul
NBLK = BC // 128          # 64 blocks of 128 rows per core

_cache = {}
STAGE1_CHUNKS = [512] * 16


def _f32(dt_mod):
    return dt_mod.float32


def _build_stage1(chunks=None, xbufs=4, psh_bufs=3, ps2_bufs=2, pst_bufs=2, lo_bufs=3, paired=False):
    import concourse.bacc as bacc
    import concourse.mybir as mybir
    from concourse.tile import TileContext
    from concourse.masks import make_identity

    F16 = mybir.dt.float16
    F32 = mybir.dt.float32

    nc = bacc.Bacc("TRN2", target_bir_lowering=False, debug=False, num_devices=NCORES)
    xh = nc.dram_tensor("xh", [D * BC], F16, kind="ExternalInput")
    F8 = mybir.dt.float8e4
    xl = nc.dram_tensor("xl", [D * BC], F8, kind="ExternalInput")
    wp = nc.dram_tensor("wp", [KP, KC * 84], F16, kind="ExternalInput")
    F8E4 = mybir.dt.float8e4
    wp8 = nc.dram_tensor("wp8", [KP, KC * 16], F8E4, kind="ExternalInput")
    hb = nc.dram_tensor("hb", [128, NBLK * 10], F32, kind="ExternalOutput")
    mom = nc.dram_tensor("mom", [10, 11], F32, kind="ExternalOutput")

    with TileContext(nc) as tc:
        with (
            tc.tile_pool(name="const", bufs=1) as cpool,
            tc.tile_pool(name="xs", bufs=xbufs) as xpool,
            tc.tile_pool(name="hts", bufs=1) as hpool,
            tc.tile_pool(name="ps_h", bufs=psh_bufs, space="PSUM") as ps_h,
            tc.tile_pool(name="ps_2", bufs=ps2_bufs, space="PSUM") as ps_2,
            tc.tile_pool(name="ps_t", bufs=pst_bufs, space="PSUM") as ps_t,
            tc.tile_pool(name="ps_s", bufs=1, space="PSUM") as ps_s,
        ):
            wp_sb = cpool.tile([KP, KC * 84], F16, name="wp_sb")
            nc.sync.dma_start(wp_sb[:], wp[:])
            wp8_sb = cpool.tile([KP, KC, 16], F8E4, name="wp8_sb")
            nc.sync.dma_start(
                wp8_sb[:], wp8[:].rearrange("p (k m) -> p k m", k=KC)
            )
            ident = cpool.tile([10, 10], F32, name="ident")
            make_identity(nc, ident[:])

            ht_sb = hpool.tile([10, BC], F32, name="ht_sb")
            hn_sb = hpool.tile([128, NBLK * 10], F32, name="hn_sb")
            s1p = hpool.tile([10, 24], F32, name="s1p")
            mom_sb = hpool.tile([10, 11], F32, name="mom_sb")

            ps_S = ps_s.tile([10, 10], F32, name="ps_S")

            nblk_per_tile = NT // 128  # 4
            blk = 0
            pending = None

            def emit_sblock(b0, nb):
                # nb transposes share one psum tile, drained by one copy
                pt = ps_t.tile([128, 4 * 10], F32, tag="pt", name="pt")
                for bb in range(nb):
                    nc.tensor.transpose(
                        pt[:, bb * 10:(bb + 1) * 10],
                        ht_sb[:, (b0 + bb) * 128:(b0 + bb + 1) * 128],
                        ident[:],
                    )
                nc.vector.tensor_copy(
                    hn_sb[:, b0 * 10:(b0 + nb) * 10], pt[:, 0:nb * 10]
                )
                for bb in range(nb):
                    b2 = b0 + bb
                    nc.tensor.matmul(
                        ps_S[:],
                        hn_sb[:, b2 * 10:(b2 + 1) * 10],
                        hn_sb[:, b2 * 10:(b2 + 1) * 10],
                        start=(b2 == 0),
                        stop=(b2 == NBLK - 1),
                    )
            # ramp-up / ramp-down chunk widths: small first chunk lets PE start
            # early; small last chunks shrink the post-DMA tail
            CHUNKS = chunks or STAGE1_CHUNKS
            assert sum(CHUNKS) == BC
            off = 0
            pos = 0
            for ob, W in enumerate(CHUNKS):
                # one DMA per plane per chunk: dest covers all 7 k-slabs
                xh_t = xpool.tile([KP, KC, CB], F16, tag="xh", name="xh_t")
                nc.sync.dma_start(
                    xh_t[:, :, 0:W],
                    xh[pos:pos + KP * KC * W].rearrange(
                        "(p k w) -> p k w", p=KP, k=KC
                    ),
                )
                xl_t = xpool.tile([KP, KC, CB], F8, tag="xl", name="xl_t")
                nc.sync.dma_start(
                    xl_t[:, :, 0:W],
                    xl[pos:pos + KP * KC * W].rearrange(
                        "(p k w) -> p k w", p=KP, k=KC
                    ),
                )
                pos += KP * KC * W
                for j in range((W + NT - 1) // NT):
                    n = min(NT, W - j * NT)
                    ps = ps_h.tile([128, NT], F32, tag="ps", name="ps")
                    ps2 = ps_2.tile([10, NT], F32, tag="ps2", name="ps2")
                    # hi pass: fp16, psum[0:10] = xh@Wh, psum[32:42] = xh@Wl*2^6
                    for k in range(KC):
                        nc.tensor.matmul(
                            ps[0:42, 0:n],
                            wp_sb[:, k * 84:k * 84 + 42],
                            xh_t[:, k, j * NT:j * NT + n],
                            start=(k == 0),
                            stop=(k == KC - 1),
                        )
                    # lo pass: fp8e4 DoubleRow pairs two 112-row chunks per
                    # matmul; psum[64:74] = e@W * 2^16
                    for kp in range(0, KC - 1, 2):
                        nc.tensor.matmul(
                            ps2[:, 0:n],
                            wp8_sb[:, kp:kp + 2, 0:10],
                            xl_t[:, kp:kp + 2, j * NT:j * NT + n],
                            start=(kp == 0),
                            stop=False,
                            perf_mode=mybir.MatmulPerfMode.DoubleRow,
                        )
                    nc.tensor.matmul(
                        ps2[:, 0:n],
                        wp8_sb[:, KC - 1, 0:10],
                        xl_t[:, KC - 1, j * NT:j * NT + n],
                        start=False,
                        stop=True,
                    )
                    col0 = off + j * NT
                    lo_t = hpool.tile([10, NT], F32, tag="lo", bufs=lo_bufs, name="lo_t")
                    nc.scalar.activation(
                        lo_t[:, 0:n], ps[32:42, 0:n],
                        mybir.ActivationFunctionType.Copy, scale=2.0 ** -6,
                    )
                    lo_u = hpool.tile([10, NT], F32, tag="lou", bufs=lo_bufs, name="lo_u")
                    nc.scalar.activation(
                        lo_u[:, 0:n], ps2[:, 0:n],
                        mybir.ActivationFunctionType.Copy, scale=2.0 ** -16,
                    )
                    nc.vector.tensor_add(lo_t[:, 0:n], lo_t[:, 0:n], lo_u[:, 0:n])
                    nc.vector.tensor_add(
                        ht_sb[:, col0:col0 + n], ps[0:10, 0:n], lo_t[:, 0:n]
                    )
                    if pending is not None:
                        emit_sblock(*pending)
                    pending = (blk, n // 128)
                    blk += n // 128
                nc.vector.reduce_sum(
                    s1p[:, ob:ob + 1],
                    ht_sb[:, off:off + W],
                    axis=mybir.AxisListType.X,
                )
                off += W
            if pending is not None:
                emit_sblock(*pending)
            nc.vector.reduce_sum(
                mom_sb[:, 0:1], s1p[:, 0:len(CHUNKS)],
                axis=mybir.AxisListType.X,
            )
            nc.vector.tensor_copy(mom_sb[:, 1:11], ps_S[:])
            # 3-way split: only the last 4 blocks' piece sits on the tail
            c1 = NBLK * 10 // 2            # blocks 0-31
            c2 = (NBLK - 4) * 10           # blocks 32-59
            nc.sync.dma_start(hb[:, 0:c1], hn_sb[:, 0:c1])
            nc.sync.dma_start(hb[:, c1:c2], hn_sb[:, c1:c2])
            nc.sync.dma_start(hb[:, c2:], hn_sb[:, c2:])
            nc.sync.dma_start(mom[:], mom_sb[:])
    nc.finalize()
    return nc


def _build_stage2():
    import concourse.bacc as bacc
    import concourse.mybir as mybir
    from concourse.tile import TileContext

    F16 = mybir.dt.float16
    F32 = mybir.dt.float32

    nc = bacc.Bacc("TRN2", target_bir_lowering=False, debug=False, num_devices=NCORES)
    hp = nc.dram_tensor("hp", [33, BC], F16, kind="ExternalInput")
    tm = nc.dram_tensor("tm", [33, 10], F16, kind="ExternalInput")
    ob = nc.dram_tensor("ob", [128, NBLK * 10], F32, kind="ExternalOutput")

    with TileContext(nc) as tc:
        with (
            tc.tile_pool(name="sb", bufs=1) as sb,
            tc.tile_pool(name="ps", bufs=4, space="PSUM") as psp,
        ):
            tm_sb = sb.tile([33, 10], F16, name="tm_sb")
            nc.sync.dma_start(tm_sb[:], tm[:])
            hp_sb = sb.tile([33, BC], F16, name="hp_sb")
            half = BC // 2
            for p in range(2):
                nc.sync.dma_start(
                    hp_sb[:, p * half:(p + 1) * half],
                    hp[:, p * half:(p + 1) * half],
                )
            ob_sb = sb.tile([128, NBLK * 10], F32, name="ob_sb")
            GRP = 16
            for g0 in range(NBLK // GRP):
                ps = psp.tile([128, GRP * 10], F32, tag="ps", name="ps")
                for bb in range(GRP):
                    b = g0 * GRP + bb
                    nc.tensor.matmul(
                        ps[:, bb * 10:(bb + 1) * 10],
                        hp_sb[:, b * 128:(b + 1) * 128],
                        tm_sb[:],
                        start=True,
                        stop=True,
                    )
                nc.vector.tensor_copy(
                    ob_sb[:, g0 * GRP * 10:(g0 + 1) * GRP * 10], ps[:]
                )
            nc.sync.dma_start(ob[:], ob_sb[:])
    nc.finalize()
    return nc


def _chain_host(s1, S, W0, b0, g0, beta0, Ws, bs, gs, betas, Wf, bf):
    """Collapse BN chain on global moments of h = x@W0.T (no bias). float64.
    Returns Tmat [10,10], r [10] with out = h @ Tmat + r."""
    m = s1.astype(np.float64) / B
    C = S.astype(np.float64) / B - np.outer(m, m)
    g0 = g0.astype(np.float64)
    var0 = np.diag(C).copy()
    A = np.diag(g0 / np.sqrt(var0 + EPS))
    d = beta0.astype(np.float64).copy()
    Ws64 = Ws.astype(np.float64)
    gs64 = gs.astype(np.float64)
    betas64 = betas.astype(np.float64)
    for k in range(Ws64.shape[0]):
        Ak = A @ Ws64[k].T
        var = np.einsum("ij,ik,kj->j", Ak, C, Ak)
        A = Ak * (gs64[k] / np.sqrt(var + EPS))[None, :]
        d = betas64[k].copy()
    Tmat = A @ Wf.astype(np.float64).T
    r = d @ Wf.astype(np.float64).T + bf.astype(np.float64)
    # fold bias b0 and centering: out = (h + b0 - (m + b0)) @ Tmat + r
    return Tmat, (r - m @ Tmat)


def _split16(a):
    hi = a.astype(np.float16)
    lo = (a.astype(np.float32) - hi.astype(np.float32)).astype(np.float16)
    return hi, lo


def kernel(**inputs):
    from concourse.bass_utils import run_bass_kernel_spmd

    inputs = {k: np.asarray(v, dtype=np.float32) for k, v in inputs.items()}
    x = inputs["x"]
    W0 = inputs["W0"]

    if "nc1" not in _cache:
        _cache["nc1"] = _build_stage1(chunks=STAGE1_CHUNKS)
    if "nc2" not in _cache:
        _cache["nc2"] = _build_stage2()

    # ---- host marshalling for stage 1 ----
    import ml_dtypes
    F8 = ml_dtypes.float8_e4m3
    xh = x.astype(np.float16)                 # [B, D]
    e = x - xh.astype(np.float32)             # exact residual
    xl8 = (e * 4096.0).astype(F8)             # fp8e3, scale 2^12 (|.| <= ~11)
    xh_t = xh.T                               # [D, B] strided views
    xl_t = xl8.T
    W0h, W0l = _split16(W0)                   # [10, D]
    # hi-pass stationary (cols k*84..k*84+41):  W0h at +0..9, W0l*2^6 at +32..41
    # lo-pass stationary (cols k*84+42..+83):   zeros at +0..9, W0h*2^-6 at +32..41
    # psum[0:10] = xh@W0h ; psum[32:42] = (xh@W0l + e@W0h) * 2^6
    # ht = psum[0:10] + 2^-6 * psum[32:42]  (ACT applies the 2^-6 on its copy)
    wph = W0h.T.reshape(KC, KP, 10).transpose(1, 0, 2)   # [112, 7, 10]
    wpl = (W0l.astype(np.float32) * 2.0 ** 6).astype(np.float16)
    wpl = wpl.T.reshape(KC, KP, 10).transpose(1, 0, 2)
    wph_dn = (W0h.astype(np.float32) * 2.0 ** -6).astype(np.float16)
    wph_dn = wph_dn.T.reshape(KC, KP, 10).transpose(1, 0, 2)
    wp = np.zeros((KP, KC, 84), dtype=np.float16)
    wp[:, :, 0:10] = wph
    wp[:, :, 32:42] = wpl
    wp[:, :, 74:84] = wph_dn
    wp = np.ascontiguousarray(wp.reshape(KP, KC * 84))
    # fp8e4 lo-pass stationary: full W * 2^4 (psum term lands at scale 2^16)
    w8 = (W0.astype(np.float32) * 2.0 ** 4).astype(F8)
    w8 = w8.T.reshape(KC, KP, 10).transpose(1, 0, 2)     # [112, 7, 10]
    wp8 = np.zeros((KP, KC, 16), dtype=F8)
    wp8[:, :, 0:10] = w8
    wp8 = np.ascontiguousarray(wp8.reshape(KP, KC * 16))

    CHUNKS = STAGE1_CHUNKS
    in1 = []
    for c in range(NCORES):
        sl = slice(c * BC, (c + 1) * BC)
        xhc = np.ascontiguousarray(xh_t[:, sl])      # [784, 8192] fp16
        xlc = np.ascontiguousarray(xl_t[:, sl])      # [784, 8192] fp8
        hblob = np.empty(D * BC, dtype=np.float16)
        lblob = np.empty(D * BC, dtype=xlc.dtype)
        h3 = xhc.reshape(KC, KP, BC)
        l3 = xlc.reshape(KC, KP, BC)
        pos = 0
        off = 0
        for W in CHUNKS:
            n = KP * KC * W
            hblob[pos:pos + n] = h3[:, :, off:off + W].transpose(1, 0, 2).ravel()
            lblob[pos:pos + n] = l3[:, :, off:off + W].transpose(1, 0, 2).ravel()
            pos += n
            off += W
        in1.append({"xh": hblob, "xl": lblob, "wp": wp, "wp8": wp8})
    res1 = run_bass_kernel_spmd(_cache["nc1"], in1, core_ids=list(range(NCORES)))

    # ---- gather moments, run the tiny chain on host ----
    s1 = np.zeros(10, dtype=np.float64)
    S = np.zeros((10, 10), dtype=np.float64)
    h_parts = []
    for c in range(NCORES):
        mom = np.asarray(res1.results[c]["mom"], dtype=np.float64)
        s1 += mom[:, 0]
        S += mom[:, 1:11]
        hbc = np.asarray(res1.results[c]["hb"])          # [128, 640]
        h_parts.append(hbc.reshape(128, NBLK, 10).transpose(1, 0, 2).reshape(BC, 10))
    h = np.concatenate(h_parts, axis=0)                   # [B, 10] fp32

    Tmat, r = _chain_host(
        s1, S,
        W0, inputs["b0"], inputs["g0"], inputs["beta0"],
        inputs["Ws"], inputs["bs"], inputs["gs"], inputs["betas"],
        inputs["Wf"], inputs["bf"],
    )

    # ---- host marshalling for stage 2 ----
    Tb = np.concatenate([Tmat, r[None, :]], axis=0).astype(np.float32)  # [11, 10]
    Tbh, Tbl = _split16(Tb)
    # K=33 pairing: [hth;1]@Tbh + [htl;0]@Tbh + [hth;1]@Tbl
    tmv = np.concatenate([Tbh, Tbh, Tbl], axis=0)        # [33, 10] fp16

    ht = h.T                                              # [10, B] fp32 view
    hth, htl = _split16(ht)                               # [10, B] fp16
    in2 = []
    for c in range(NCORES):
        sl = slice(c * BC, (c + 1) * BC)
        hpc = np.zeros((33, BC), dtype=np.float16)
        hpc[0:10] = hth[:, sl]
        hpc[10, :] = 1.0
        hpc[11:21] = htl[:, sl]
        hpc[22:32] = hth[:, sl]
        hpc[32, :] = 1.0
        in2.append({"hp": hpc, "tm": tmv})
    res2 = run_bass_kernel_spmd(_cache["nc2"], in2, core_ids=list(range(NCORES)))

    out_parts = []
    for c in range(NCORES):
        obc = np.asarray(res2.results[c]["ob"])           # [128, 640]
        out_parts.append(obc.reshape(128, NBLK, 10).transpose(1, 0, 2).reshape(BC, 10))
    return np.ascontiguousarray(np.concatenate(out_parts, axis=0))



# revision 11
# speedup vs baseline: 2.3105x; 2.3105x over previous
"""Trainium2 Bass kernel for nn_DeepLinear (784->10 linear + BN, 62x(10->10 linear + BN), 10->10 linear).

Math: BN output has exact per-column batch mean beta, so every layer past the
first acts linearly on the *centered* activations. The whole net collapses to:
    h  = x @ W0.T                      (heavy, on device, data-parallel over batch)
    mu = mean(h), S = h'^T h'          (global batch moments; per-core partial
                                        moments combined on host = the sync-BN
                                        all-reduce)
    T, r = 62-layer chain of 10x10 covariance algebra (tiny, host, float64)
    out = h @ T + r                    (light, on device)

Stage 1 streams x as fp8e3m4 (1 byte/elem - the DMA floor, ~17.8us/core) and
runs the matmul weight-stationary per 128-row batch block: lhsT = x-block
[112, 128] (fp8), rhs = W0 chunk [112, 10] (fp16), accumulating h blocks
[128, 10] in PSUM over the 7 feature chunks. h is rounded to fp16 (error
negligible vs fp8 x); an appended ones-column turns one [128,11]x[128,11]
matmul per block into the moment accumulator ([11,11] = [S, s; s^T, n]).
The moment matmuls stop at block 55 so the mom result rides inside the hb
tensor (bitcast fp32 region) and nothing moment-related sits on the DMA tail;
the host tops up S/s1 with the last 8 blocks from the same fp16 h values
(bit-identical math to the device matmul path).

Stage 2 applies the collapsed affine map with h^T blocks stationary and
Tb = [T; r] (fp16) moving: one matmul per 128-row block. Tb rides inside the
hp input tensor. All input/output DMAs are split so x/compute DMAs issue from
the SP queue and mid-stream output DMAs from the ACT queue (they don't block
each other's SEQ), with only a minimal last piece on the tail.
"""

import numpy as np

EPS = 1e-5
B = 65536
D = 784
NCORES = 8
BC = B // NCORES          # 8192 rows per core
KP = 112                  # contraction chunk partitions (7 * 112 = 784)
KC = 7                    # contraction chunks
NBLK = BC // 128          # 64 blocks of 128 rows per core
MOMBLK = 56               # blocks with device-side moment accumulation
HBW = NBLK * 11 + 24      # hb width: 64 blocks * 11 cols + mom region (22) + pad

_cache = {}
# batch-column widths of the stage-1 x DMA chunks (must sum to BC); the small
# trailing chunks shrink the post-DMA compute tail
STAGE1_CHUNKS = [1024] * 7 + [512, 512]


def _build_stage1(chunks=None):
    import concourse.bacc as bacc
    import concourse.mybir as mybir
    from concourse.tile import TileContext

    F16 = mybir.dt.float16
    F32 = mybir.dt.float32
    F8E3 = mybir.dt.float8e3

    CHUNKS = chunks or STAGE1_CHUNKS
    assert sum(CHUNKS) == BC
    CBMAX = max(CHUNKS)

    nc = bacc.Bacc("TRN2", target_bir_lowering=False, debug=False, num_devices=NCORES)
    x8 = nc.dram_tensor("x8", [KP, KC, BC], F8E3, kind="ExternalInput")
    w16 = nc.dram_tensor("w16", [KP, KC * 10], F16, kind="ExternalInput")
    hb = nc.dram_tensor("hb", [128, HBW], F16, kind="ExternalOutput")

    with TileContext(nc) as tc:
        with (
            tc.tile_pool(name="const", bufs=1) as cpool,
            tc.tile_pool(name="xs", bufs=3) as xpool,
            tc.tile_pool(name="hts", bufs=1) as hpool,
            tc.tile_pool(name="ps_h", bufs=3, space="PSUM") as ps_h,
            tc.tile_pool(name="ps_m", bufs=1, space="PSUM") as ps_m,
        ):
            w_sb = cpool.tile([KP, KC, 10], F16, name="w_sb")
            hn_sb = hpool.tile([128, HBW], F16, name="hn_sb")
            hn3 = hn_sb[:, 0:NBLK * 11].rearrange("p (b c) -> p b c", c=11)
            mom_sb = hn_sb[0:11, NBLK * 11:NBLK * 11 + 22].bitcast(F32)
            ps_mom = ps_m.tile([11, 11], F32, name="ps_mom")

            # hb[0:616] (blocks 0..55) is issued after the last x chunk on the
            # ACT queue: its transfer lands in the DMA idle window while the
            # tail chunk's compute drains, off the x stream entirely
            hb_mid = (0, 56 * 11)

            blk = 0
            off = 0
            first = True
            for ci, W in enumerate(CHUNKS):
                x_t = xpool.tile([KP, KC, CBMAX], F8E3, tag="x", name="x_t")
                nc.sync.dma_start(x_t[:, :, 0:W], x8[:, :, off:off + W])
                if first:
                    # w issues after x0 so x0 wins the first HWDGE slot
                    nc.sync.dma_start(
                        w_sb[:], w16[:].rearrange("p (k m) -> p k m", k=KC)
                    )
                    nc.vector.memset(hn3[:, :, 10:11], 1.0)
                    first = False
                nb = W // 128
                ps = ps_h.tile([128, nb * 10], F32, tag="ps", name="ps")
                for j in range(nb):
                    for k in range(KC):
                        nc.tensor.matmul(
                            ps[:, j * 10:(j + 1) * 10],
                            x_t[:, k, j * 128:(j + 1) * 128],
                            w_sb[:, k, :],
                            start=(k == 0),
                            stop=(k == KC - 1),
                        )
                # h block columns (skip the ones columns) in one strided copy
                nc.vector.tensor_copy(
                    hn3[:, blk:blk + nb, 0:10],
                    ps[:].rearrange("p (b c) -> p b c", c=10),
                )
                for j in range(nb):
                    b = blk + j
                    if b < MOMBLK:
                        nc.tensor.matmul(
                            ps_mom[:],
                            hn3[:, b, :],
                            hn3[:, b, :],
                            start=(b == 0),
                            stop=(b == MOMBLK - 1),
                        )
                blk += nb
                off += W
                if blk == MOMBLK:
                    nc.vector.tensor_copy(mom_sb, ps_mom[:])
                if ci in hb_pieces:
                    c0, c1 = hb_pieces[ci]
                    nc.scalar.dma_start(hb[:, c0:c1], hn_sb[:, c0:c1])
            # tail: blocks 56..63 plus the mom bytes, one small DMA on SP
            nc.sync.dma_start(hb[:, 56 * 11:HBW], hn_sb[:, 56 * 11:HBW])
    nc.finalize()
    return nc


def _build_stage2():
    import concourse.bacc as bacc
    import concourse.mybir as mybir
    from concourse.tile import TileContext

    F16 = mybir.dt.float16
    F32 = mybir.dt.float32

    nc = bacc.Bacc("TRN2", target_bir_lowering=False, debug=False, num_devices=NCORES)
    # cols 0..2047: h.T cols 0..2047; 2048..2057: Tb; 2064..8207: h.T cols 2048..
    HPW = BC + 16
    hp = nc.dram_tensor("hp", [11, HPW], F16, kind="ExternalInput")
    ob = nc.dram_tensor("ob", [128, NBLK * 10], F16, kind="ExternalOutput")

    def colof(b):
        return b * 128 if b < 16 else 2064 + (b - 16) * 128

    with TileContext(nc) as tc:
        with (
            tc.tile_pool(name="sb", bufs=1) as sb,
            tc.tile_pool(name="ps", bufs=4, space="PSUM") as psp,
        ):
            hp_sb = sb.tile([11, HPW], F16, name="hp_sb")
            nc.sync.dma_start(hp_sb[:, 0:4112], hp[:, 0:4112])
            nc.sync.dma_start(hp_sb[:, 4112:HPW], hp[:, 4112:HPW])
            tb_sb = hp_sb[:, 2048:2058]
            ob_sb = sb.tile([128, NBLK * 10], F16, name="ob_sb")
            GRP = 16
            for g0 in range(NBLK // GRP):
                ps = psp.tile([128, GRP * 10], F32, tag="ps", name="ps")
                for bb in range(GRP):
                    b = g0 * GRP + bb
                    nc.tensor.matmul(
                        ps[:, bb * 10:(bb + 1) * 10],
                        hp_sb[:, colof(b):colof(b) + 128],
                        tb_sb,
                        start=True,
                        stop=True,
                    )
                dst = ob_sb[:, g0 * GRP * 10:(g0 + 1) * GRP * 10]
                if g0 % 2 == 0:
                    nc.vector.tensor_copy(dst, ps[:])
                else:
                    nc.scalar.activation(
                        dst, ps[:], mybir.ActivationFunctionType.Copy
                    )
            nc.sync.dma_start(ob[:], ob_sb[:])
    nc.finalize()
    return nc


def _chain_host(s1, S, W0, b0, g0, beta0, Ws, bs, gs, betas, Wf, bf):
    """Collapse BN chain on global moments of h = x@W0.T (no bias). float64.
    Returns Tmat [10,10], r [10] with out = h @ Tmat + r."""
    m = s1.astype(np.float64) / B
    C = S.astype(np.float64) / B - np.outer(m, m)
    g0 = g0.astype(np.float64)
    var0 = np.diag(C).copy()
    A = np.diag(g0 / np.sqrt(var0 + EPS))
    d = beta0.astype(np.float64).copy()
    Ws64 = Ws.astype(np.float64)
    gs64 = gs.astype(np.float64)
    betas64 = betas.astype(np.float64)
    for k in range(Ws64.shape[0]):
        Ak = A @ Ws64[k].T
        var = np.einsum("ij,ik,kj->j", Ak, C, Ak)
        A = Ak * (gs64[k] / np.sqrt(var + EPS))[None, :]
        d = betas64[k].copy()
    Tmat = A @ Wf.astype(np.float64).T
    r = d @ Wf.astype(np.float64).T + bf.astype(np.float64)
    # fold bias b0 and centering: out = (h + b0 - (m + b0)) @ Tmat + r
    return Tmat, (r - m @ Tmat)


def kernel(**inputs):
    from concourse.bass_utils import run_bass_kernel_spmd
    import ml_dtypes

    E3 = ml_dtypes.float8_e3m4

    inputs = {k: np.asarray(v, dtype=np.float32) for k, v in inputs.items()}
    x = inputs["x"]
    W0 = inputs["W0"]

    if "nc1" not in _cache:
        _cache["nc1"] = _build_stage1(chunks=STAGE1_CHUNKS)
    if "nc2" not in _cache:
        _cache["nc2"] = _build_stage2()

    # ---- host marshalling for stage 1 ----
    x8 = x.astype(E3)                                    # [B, D] 1 byte/elem
    # w blob [112, 7, 10]: chunk k = features k*112 .. k*112+111
    wb = np.ascontiguousarray(
        W0.T.reshape(KC, KP, 10).transpose(1, 0, 2).reshape(KP, KC * 10)
    ).astype(np.float16)

    in1 = []
    for c in range(NCORES):
        sl = slice(c * BC, (c + 1) * BC)
        xc = np.ascontiguousarray(
            x8[sl].T.reshape(KC, KP, BC).transpose(1, 0, 2)
        )                                                 # [112, 7, 8192] fp8
        in1.append({"x8": xc, "w16": wb})
    res1 = run_bass_kernel_spmd(_cache["nc1"], in1, core_ids=list(range(NCORES)))

    # ---- gather moments (device blocks 0..55 + host top-up 56..63), chain ----
    s1 = np.zeros(10, dtype=np.float64)
    S = np.zeros((10, 10), dtype=np.float64)
    h_parts = []
    for c in range(NCORES):
        hbc = np.asarray(res1.results[c]["hb"])                    # [128, HBW] fp16
        mom = np.ascontiguousarray(hbc[0:11, NBLK * 11:NBLK * 11 + 22]).view(
            np.float32
        ).astype(np.float64)                                       # [11, 11]
        s1 += mom[10, 0:10]
        S += mom[0:10, 0:10]
        h16 = hbc[:, 0:NBLK * 11].reshape(128, NBLK, 11)[:, :, 0:10]
        tail = h16[:, MOMBLK:, :].astype(np.float64).reshape(-1, 10)
        S += tail.T @ tail
        s1 += tail.sum(axis=0)
        h_parts.append(h16)

    Tmat, r = _chain_host(
        s1, S,
        W0, inputs["b0"], inputs["g0"], inputs["beta0"],
        inputs["Ws"], inputs["bs"], inputs["gs"], inputs["betas"],
        inputs["Wf"], inputs["bf"],
    )

    # ---- host marshalling for stage 2 ----
    tbv = np.concatenate([Tmat, r[None, :]], axis=0).astype(np.float16)  # [11, 10]
    in2 = []
    for c in range(NCORES):
        ht = h_parts[c].transpose(1, 0, 2).reshape(BC, 10).T       # [10, BC] fp16
        hpc = np.empty((11, BC + 16), dtype=np.float16)
        hpc[0:10, 0:2048] = ht[:, 0:2048]
        hpc[10, 0:2048] = 1.0
        hpc[0:11, 2048:2058] = tbv
        hpc[:, 2058:2064] = 0
        hpc[0:10, 2064:] = ht[:, 2048:]
        hpc[10, 2064:] = 1.0
        in2.append({"hp": hpc})
    res2 = run_bass_kernel_spmd(_cache["nc2"], in2, core_ids=list(range(NCORES)))

    out_parts = []
    for c in range(NCORES):
        obc = np.asarray(res2.results[c]["ob"])           # [128, 640] fp16
        out_parts.append(
            obc.reshape(128, NBLK, 10).transpose(1, 0, 2).reshape(BC, 10)
        )
    return np.ascontiguousarray(
        np.concatenate(out_parts, axis=0).astype(np.float32)
    )


# revision 21
# speedup vs baseline: 2.3456x; 1.0152x over previous
"""Trainium2 Bass kernel for nn_DeepLinear (784->10 linear + BN, 62x(10->10 linear + BN), 10->10 linear).

Math: BN output has exact per-column batch mean beta, so every layer past the
first acts linearly on the *centered* activations. The whole net collapses to:
    h  = x @ W0.T                      (heavy, on device, data-parallel over batch)
    mu = mean(h), S = h'^T h'          (global batch moments; per-core partial
                                        moments combined on host = the sync-BN
                                        all-reduce)
    T, r = 62-layer chain of 10x10 covariance algebra (tiny, host, float64)
    out = h @ T + r                    (light, on device)

Stage 1 streams x as fp8e3m4 (1 byte/elem - the DMA floor, ~17.8us/core) and
runs the matmul weight-stationary per 128-row batch block: lhsT = x-block
[112, 128] (fp8), rhs = W0 chunk [112, 10] (fp16), accumulating h blocks
[128, 10] in PSUM over the 7 feature chunks. h is rounded to fp16 (error
negligible vs fp8 x); an appended ones-column turns one [128,11]x[128,11]
matmul per block into the moment accumulator ([11,11] = [S, s; s^T, n]).
The moment matmuls stop at block 55 so the mom result rides inside the hb
tensor (bitcast fp32 region) and nothing moment-related sits on the DMA tail;
the host tops up S/s1 with the last 8 blocks from the same fp16 h values
(bit-identical math to the device matmul path).

Stage 2 applies the collapsed affine map with h^T blocks stationary and
Tb = [T; r] (fp16) moving: one matmul per 128-row block. Tb rides inside the
hp input tensor. All input/output DMAs are split so x/compute DMAs issue from
the SP queue and mid-stream output DMAs from the ACT queue (they don't block
each other's SEQ), with only a minimal last piece on the tail.
"""

import numpy as np

EPS = 1e-5
B = 65536
D = 784
NCORES = 8
BC = B // NCORES          # 8192 rows per core
KP = 112                  # contraction chunk partitions (7 * 112 = 784)
KC = 7                    # contraction chunks
NBLK = BC // 128          # 64 blocks of 128 rows per core
MOMBLK = 56               # blocks with device-side moment accumulation
HBW = NBLK * 11 + 24      # hb width: 64 blocks * 11 cols + mom region (22) + pad

_cache = {}
# batch-column widths of the stage-1 x DMA chunks (must sum to BC); the small
# trailing chunks shrink the post-DMA compute tail. The host blob stores each
# chunk contiguously so every DMA keeps >=512B descriptors (no 2x penalty).
STAGE1_CHUNKS = [1024] * 7 + [512, 384, 128]
WCOLS = 20                # fp8 columns appended to chunk 0 carrying W0 (fp16)


def _build_stage1(chunks=None):
    import concourse.bacc as bacc
    import concourse.mybir as mybir
    from concourse.tile import TileContext

    F16 = mybir.dt.float16
    F32 = mybir.dt.float32
    F8E3 = mybir.dt.float8e3

    CHUNKS = chunks or STAGE1_CHUNKS
    assert sum(CHUNKS) == BC
    CBMAX = max(CHUNKS)

    nc = bacc.Bacc("TRN2", target_bir_lowering=False, debug=False, num_devices=NCORES)
    # flat blob: chunk 0 as [112, 7, W0+WCOLS] (x cols + W0-fp16-as-fp8 bytes),
    # then each later chunk as [112, 7, Wc], all contiguous
    XTOT = KP * KC * (BC + WCOLS)
    x8 = nc.dram_tensor("x8", [XTOT], F8E3, kind="ExternalInput")
    hb = nc.dram_tensor("hb", [128, HBW], F16, kind="ExternalOutput")

    with TileContext(nc) as tc:
        with (
            tc.tile_pool(name="const", bufs=1) as cpool,
            tc.tile_pool(name="xs", bufs=3) as xpool,
            tc.tile_pool(name="hts", bufs=1) as hpool,
            tc.tile_pool(name="ps_h", bufs=3, space="PSUM") as ps_h,
            tc.tile_pool(name="ps_m", bufs=1, space="PSUM") as ps_m,
        ):
            hn_sb = hpool.tile([128, HBW], F16, name="hn_sb")
            hn3 = hn_sb[:, 0:NBLK * 11].rearrange("p (b c) -> p b c", c=11)
            mom_sb = hn_sb[0:11, NBLK * 11:NBLK * 11 + 22].bitcast(F32)
            ps_mom = ps_m.tile([11, 11], F32, name="ps_mom")

            # hb[0:660] (blocks 0..59) is issued after the last x chunk on the
            # ACT queue: its transfer lands in the DMA idle window while the
            # tail chunk's compute drains, off the x stream entirely
            hb_mid = (0, 60 * 11)

            w_sb = None
            blk = 0
            pos = 0
            first = True
            for ci, W in enumerate(CHUNKS):
                Wd = W + (WCOLS if first else 0)
                if first:
                    x_t = cpool.tile([KP, KC, Wd], F8E3, name="x0_t")
                elif W == CBMAX:
                    x_t = xpool.tile([KP, KC, CBMAX], F8E3, tag="x", name="x_t")
                else:
                    # exact-width tile keeps the DMA descriptor elem size at
                    # KC*W contiguous bytes (no <512B 2x penalty)
                    x_t = cpool.tile([KP, KC, W], F8E3, name=f"xtail{ci}")
                n = KP * KC * Wd
                nc.sync.dma_start(
                    x_t[:, :, 0:Wd],
                    x8[pos:pos + n].rearrange("(p k w) -> p k w", p=KP, k=KC),
                )
                pos += n
                if first:
                    w_sb = x_t[:, :, W:W + WCOLS].bitcast(F16)  # [112, 7, 10]
                    nc.vector.memset(hn3[:, :, 10:11], 1.0)
                    first = False
                nb = W // 128
                ps = ps_h.tile([128, nb * 10], F32, tag="ps", name="ps")
                for j in range(nb):
                    for k in range(KC):
                        nc.tensor.matmul(
                            ps[:, j * 10:(j + 1) * 10],
                            x_t[:, k, j * 128:(j + 1) * 128],
                            w_sb[:, k, :],
                            start=(k == 0),
                            stop=(k == KC - 1),
                        )
                # h block columns (skip the ones columns) in one strided copy
                nc.vector.tensor_copy(
                    hn3[:, blk:blk + nb, 0:10],
                    ps[:].rearrange("p (b c) -> p b c", c=10),
                )
                for j in range(nb):
                    b = blk + j
                    if b < MOMBLK:
                        nc.tensor.matmul(
                            ps_mom[:],
                            hn3[:, b, :],
                            hn3[:, b, :],
                            start=(b == 0),
                            stop=(b == MOMBLK - 1),
                        )
                blk += nb
                if blk == MOMBLK:
                    nc.vector.tensor_copy(mom_sb, ps_mom[:])
                if ci == len(CHUNKS) - 1:
                    nc.scalar.dma_start(
                        hb[:, hb_mid[0]:hb_mid[1]], hn_sb[:, hb_mid[0]:hb_mid[1]]
                    )
            # tail: blocks 56..63 plus the mom bytes, one small DMA on SP
            nc.sync.dma_start(hb[:, 56 * 11:HBW], hn_sb[:, 56 * 11:HBW])
    nc.finalize()
    return nc


def _build_stage2():
    import concourse.bacc as bacc
    import concourse.mybir as mybir
    from concourse.tile import TileContext

    F16 = mybir.dt.float16
    F32 = mybir.dt.float32

    nc = bacc.Bacc("TRN2", target_bir_lowering=False, debug=False, num_devices=NCORES)
    # cols 0..8191: h.T; 8192..8201: Tb = [T; r]
    HPW = BC + 10
    hp = nc.dram_tensor("hp", [11, HPW], F16, kind="ExternalInput")
    ob = nc.dram_tensor("ob", [128, NBLK * 10], F16, kind="ExternalOutput")

    with TileContext(nc) as tc:
        with (
            tc.tile_pool(name="sb", bufs=1) as sb,
            tc.tile_pool(name="ps", bufs=2, space="PSUM") as psp,
        ):
            hp_sb = sb.tile([11, HPW], F16, name="hp_sb")
            nc.sync.dma_start(hp_sb[:], hp[:])
            tb_sb = hp_sb[:, BC:BC + 10]
            ob_sb = sb.tile([128, NBLK * 10], F16, name="ob_sb")
            GRP = 32
            for g0 in range(NBLK // GRP):
                ps = psp.tile([128, GRP * 10], F32, tag="ps", name="ps")
                for bb in range(GRP):
                    b = g0 * GRP + bb
                    nc.tensor.matmul(
                        ps[:, bb * 10:(bb + 1) * 10],
                        hp_sb[:, b * 128:b * 128 + 128],
                        tb_sb,
                        start=True,
                        stop=True,
                    )
                dst = ob_sb[:, g0 * GRP * 10:(g0 + 1) * GRP * 10]
                if g0 % 2 == 0:
                    nc.vector.tensor_copy(dst, ps[:])
                else:
                    nc.scalar.activation(
                        dst, ps[:], mybir.ActivationFunctionType.Copy
                    )
            nc.sync.dma_start(ob[:], ob_sb[:])
    nc.finalize()
    return nc


def _chain_host(s1, S, W0, b0, g0, beta0, Ws, bs, gs, betas, Wf, bf):
    """Collapse BN chain on global moments of h = x@W0.T (no bias). float64.
    Returns Tmat [10,10], r [10] with out = h @ Tmat + r."""
    m = s1.astype(np.float64) / B
    C = S.astype(np.float64) / B - np.outer(m, m)
    g0 = g0.astype(np.float64)
    var0 = np.diag(C).copy()
    A = np.diag(g0 / np.sqrt(var0 + EPS))
    d = beta0.astype(np.float64).copy()
    Ws64 = Ws.astype(np.float64)
    gs64 = gs.astype(np.float64)
    betas64 = betas.astype(np.float64)
    for k in range(Ws64.shape[0]):
        Ak = A @ Ws64[k].T
        var = np.einsum("ij,ik,kj->j", Ak, C, Ak)
        A = Ak * (gs64[k] / np.sqrt(var + EPS))[None, :]
        d = betas64[k].copy()
    Tmat = A @ Wf.astype(np.float64).T
    r = d @ Wf.astype(np.float64).T + bf.astype(np.float64)
    # fold bias b0 and centering: out = (h + b0 - (m + b0)) @ Tmat + r
    return Tmat, (r - m @ Tmat)


def kernel(**inputs):
    from concourse.bass_utils import run_bass_kernel_spmd
    import ml_dtypes

    E3 = ml_dtypes.float8_e3m4

    inputs = {k: np.asarray(v, dtype=np.float32) for k, v in inputs.items()}
    x = inputs["x"]
    W0 = inputs["W0"]

    if "nc1" not in _cache:
        _cache["nc1"] = _build_stage1(chunks=STAGE1_CHUNKS)
    if "nc2" not in _cache:
        _cache["nc2"] = _build_stage2()

    # ---- host marshalling for stage 1 ----
    x8 = x.astype(E3)                                    # [B, D] 1 byte/elem
    # w [112, 7, 10] fp16 -> raw bytes as fp8 cols: chunk k = feats k*112..+111
    wb = np.ascontiguousarray(
        W0.T.reshape(KC, KP, 10).transpose(1, 0, 2)
    ).astype(np.float16)                                  # [112, 7, 10]
    wb8 = wb.view(np.uint8).reshape(KP, KC, WCOLS)        # fp16 bytes as uint8

    CHUNKS = STAGE1_CHUNKS
    XTOT = KP * KC * (BC + WCOLS)
    in1 = []
    for c in range(NCORES):
        sl = slice(c * BC, (c + 1) * BC)
        xc = np.ascontiguousarray(
            x8[sl].T.reshape(KC, KP, BC).transpose(1, 0, 2)
        )                                                 # [112, 7, 8192] fp8
        blob = np.empty(XTOT, dtype=np.uint8)
        pos = 0
        off = 0
        for ci, W in enumerate(CHUNKS):
            if ci == 0:
                seg = np.concatenate(
                    [xc[:, :, 0:W].view(np.uint8), wb8], axis=2
                )
            else:
                seg = xc[:, :, off:off + W].view(np.uint8)
            n = seg.size
            blob[pos:pos + n] = seg.ravel()
            pos += n
            off += W
        in1.append({"x8": blob.view(E3)})
    res1 = run_bass_kernel_spmd(_cache["nc1"], in1, core_ids=list(range(NCORES)))

    # ---- gather moments (device blocks 0..55 + host top-up 56..63), chain ----
    s1 = np.zeros(10, dtype=np.float64)
    S = np.zeros((10, 10), dtype=np.float64)
    h_parts = []
    for c in range(NCORES):
        hbc = np.asarray(res1.results[c]["hb"])                    # [128, HBW] fp16
        mom = np.ascontiguousarray(hbc[0:11, NBLK * 11:NBLK * 11 + 22]).view(
            np.float32
        ).astype(np.float64)                                       # [11, 11]
        s1 += mom[10, 0:10]
        S += mom[0:10, 0:10]
        h16 = hbc[:, 0:NBLK * 11].reshape(128, NBLK, 11)[:, :, 0:10]
        tail = h16[:, MOMBLK:, :].astype(np.float64).reshape(-1, 10)
        S += tail.T @ tail
        s1 += tail.sum(axis=0)
        h_parts.append(h16)

    Tmat, r = _chain_host(
        s1, S,
        W0, inputs["b0"], inputs["g0"], inputs["beta0"],
        inputs["Ws"], inputs["bs"], inputs["gs"], inputs["betas"],
        inputs["Wf"], inputs["bf"],
    )

    # ---- host marshalling for stage 2 ----
    tbv = np.concatenate([Tmat, r[None, :]], axis=0).astype(np.float16)  # [11, 10]
    in2 = []
    for c in range(NCORES):
        ht = h_parts[c].transpose(1, 0, 2).reshape(BC, 10).T       # [10, BC] fp16
        hpc = np.empty((11, BC + 10), dtype=np.float16)
        hpc[0:10, 0:BC] = ht
        hpc[10, 0:BC] = 1.0
        hpc[0:11, BC:BC + 10] = tbv
        in2.append({"hp": hpc})
    res2 = run_bass_kernel_spmd(_cache["nc2"], in2, core_ids=list(range(NCORES)))

    out_parts = []
    for c in range(NCORES):
        obc = np.asarray(res2.results[c]["ob"])           # [128, 640] fp16
        out_parts.append(
            obc.reshape(128, NBLK, 10).transpose(1, 0, 2).reshape(BC, 10)
        )
    return np.ascontiguousarray(
        np.concatenate(out_parts, axis=0).astype(np.float32)
    )


# revision 22
# speedup vs baseline: 2.3502x; 1.0019x over previous
"""Trainium2 Bass kernel for nn_DeepLinear (784->10 linear + BN, 62x(10->10 linear + BN), 10->10 linear).

Math: BN output has exact per-column batch mean beta, so every layer past the
first acts linearly on the *centered* activations. The whole net collapses to:
    h  = x @ W0.T                      (heavy, on device, data-parallel over batch)
    mu = mean(h), S = h'^T h'          (global batch moments; per-core partial
                                        moments combined on host = the sync-BN
                                        all-reduce)
    T, r = 62-layer chain of 10x10 covariance algebra (tiny, host, float64)
    out = h @ T + r                    (light, on device)

Stage 1 streams x as fp8e3m4 (1 byte/elem - the DMA floor, ~17.8us/core) and
runs the matmul weight-stationary per 128-row batch block: lhsT = x-block
[112, 128] (fp8), rhs = W0 chunk [112, 10] (fp16), accumulating h blocks
[128, 10] in PSUM over the 7 feature chunks. h is rounded to fp16 (error
negligible vs fp8 x); an appended ones-column turns one [128,11]x[128,11]
matmul per block into the moment accumulator ([11,11] = [S, s; s^T, n]).
The moment matmuls stop at block 55 so the mom result rides inside the hb
tensor (bitcast fp32 region) and nothing moment-related sits on the DMA tail;
the host tops up S/s1 with the last 8 blocks from the same fp16 h values
(bit-identical math to the device matmul path).

Stage 2 applies the collapsed affine map with h^T blocks stationary and
Tb = [T; r] (fp16) moving: one matmul per 128-row block. Tb rides inside the
hp input tensor. All input/output DMAs are split so x/compute DMAs issue from
the SP queue and mid-stream output DMAs from the ACT queue (they don't block
each other's SEQ), with only a minimal last piece on the tail.
"""

import numpy as np

EPS = 1e-5
B = 65536
D = 784
NCORES = 8
BC = B // NCORES          # 8192 rows per core
KP = 112                  # contraction chunk partitions (7 * 112 = 784)
KC = 7                    # contraction chunks
NBLK = BC // 128          # 64 blocks of 128 rows per core
MOMBLK = 56               # blocks with device-side moment accumulation
HBW = NBLK * 11 + 24      # hb width: 64 blocks * 11 cols + mom region (22) + pad

_cache = {}
# batch-column widths of the stage-1 x DMA chunks (must sum to BC); the small
# trailing chunks shrink the post-DMA compute tail. The host blob stores each
# chunk contiguously so every DMA keeps >=512B descriptors (no 2x penalty).
STAGE1_CHUNKS = [1024] * 7 + [512, 384, 128]
WCOLS = 20                # fp8 columns appended to chunk 0 carrying W0 (fp16)


def _build_stage1(chunks=None):
    import concourse.bacc as bacc
    import concourse.mybir as mybir
    from concourse.tile import TileContext

    F16 = mybir.dt.float16
    F32 = mybir.dt.float32
    F8E3 = mybir.dt.float8e3

    CHUNKS = chunks or STAGE1_CHUNKS
    assert sum(CHUNKS) == BC
    CBMAX = max(CHUNKS)

    nc = bacc.Bacc("TRN2", target_bir_lowering=False, debug=False, num_devices=NCORES)
    # flat blob: chunk 0 as [112, 7, W0+WCOLS] (x cols + W0-fp16-as-fp8 bytes),
    # then each later chunk as [112, 7, Wc], all contiguous
    XTOT = KP * KC * (BC + WCOLS)
    x8 = nc.dram_tensor("x8", [XTOT], F8E3, kind="ExternalInput")
    # two output tensors so the tail DMA has no WAW dependency on the big piece
    hba = nc.dram_tensor("hba", [128, 60 * 11], F16, kind="ExternalOutput")
    hbb = nc.dram_tensor("hbb", [128, HBW - 60 * 11], F16, kind="ExternalOutput")

    with TileContext(nc) as tc:
        with (
            tc.tile_pool(name="const", bufs=1) as cpool,
            tc.tile_pool(name="xs", bufs=3) as xpool,
            tc.tile_pool(name="hts", bufs=1) as hpool,
            tc.tile_pool(name="ps_h", bufs=3, space="PSUM") as ps_h,
            tc.tile_pool(name="ps_m", bufs=1, space="PSUM") as ps_m,
        ):
            hn_sb = hpool.tile([128, HBW], F16, name="hn_sb")
            hn3 = hn_sb[:, 0:NBLK * 11].rearrange("p (b c) -> p b c", c=11)
            mom_sb = hn_sb[0:11, NBLK * 11:NBLK * 11 + 22].bitcast(F32)
            ps_mom = ps_m.tile([11, 11], F32, name="ps_mom")

            # hb[0:660] (blocks 0..59) is issued after the last x chunk on the
            # ACT queue: its transfer lands in the DMA idle window while the
            # tail chunk's compute drains, off the x stream entirely
            hb_mid = (0, 60 * 11)

            w_sb = None
            blk = 0
            pos = 0
            first = True
            for ci, W in enumerate(CHUNKS):
                Wd = W + (WCOLS if first else 0)
                if first:
                    x_t = cpool.tile([KP, KC, Wd], F8E3, name="x0_t")
                elif W == CBMAX:
                    x_t = xpool.tile([KP, KC, CBMAX], F8E3, tag="x", name="x_t")
                else:
                    # exact-width tile keeps the DMA descriptor elem size at
                    # KC*W contiguous bytes (no <512B 2x penalty)
                    x_t = cpool.tile([KP, KC, W], F8E3, name=f"xtail{ci}")
                n = KP * KC * Wd
                nc.sync.dma_start(
                    x_t[:, :, 0:Wd],
                    x8[pos:pos + n].rearrange("(p k w) -> p k w", p=KP, k=KC),
                )
                pos += n
                if first:
                    w_sb = x_t[:, :, W:W + WCOLS].bitcast(F16)  # [112, 7, 10]
                    nc.vector.memset(hn3[:, :, 10:11], 1.0)
                    first = False
                nb = W // 128
                ps = ps_h.tile([128, nb * 10], F32, tag="ps", name="ps")
                for j in range(nb):
                    for k in range(KC):
                        nc.tensor.matmul(
                            ps[:, j * 10:(j + 1) * 10],
                            x_t[:, k, j * 128:(j + 1) * 128],
                            w_sb[:, k, :],
                            start=(k == 0),
                            stop=(k == KC - 1),
                        )
                # h block columns (skip the ones columns) in one strided copy
                nc.vector.tensor_copy(
                    hn3[:, blk:blk + nb, 0:10],
                    ps[:].rearrange("p (b c) -> p b c", c=10),
                )
                for j in range(nb):
                    b = blk + j
                    if b < MOMBLK:
                        nc.tensor.matmul(
                            ps_mom[:],
                            hn3[:, b, :],
                            hn3[:, b, :],
                            start=(b == 0),
                            stop=(b == MOMBLK - 1),
                        )
                blk += nb
                if blk == MOMBLK:
                    nc.vector.tensor_copy(mom_sb, ps_mom[:])
                if ci == len(CHUNKS) - 1:
                    nc.scalar.dma_start(
                        hba[:], hn_sb[:, hb_mid[0]:hb_mid[1]]
                    )
            # tail: blocks 60..63 plus the mom bytes, one small DMA on SP
            nc.sync.dma_start(hbb[:], hn_sb[:, 60 * 11:HBW])
    nc.finalize()
    return nc


def _build_stage2():
    import concourse.bacc as bacc
    import concourse.mybir as mybir
    from concourse.tile import TileContext

    F16 = mybir.dt.float16
    F32 = mybir.dt.float32

    nc = bacc.Bacc("TRN2", target_bir_lowering=False, debug=False, num_devices=NCORES)
    # cols 0..8191: h.T; 8192..8201: Tb = [T; r]
    HPW = BC + 10
    hp = nc.dram_tensor("hp", [11, HPW], F16, kind="ExternalInput")
    ob = nc.dram_tensor("ob", [128, NBLK * 10], F16, kind="ExternalOutput")

    with TileContext(nc) as tc:
        with (
            tc.tile_pool(name="sb", bufs=1) as sb,
            tc.tile_pool(name="ps", bufs=2, space="PSUM") as psp,
        ):
            hp_sb = sb.tile([11, HPW], F16, name="hp_sb")
            nc.sync.dma_start(hp_sb[:], hp[:])
            tb_sb = hp_sb[:, BC:BC + 10]
            ob_sb = sb.tile([128, NBLK * 10], F16, name="ob_sb")
            GRP = 32
            for g0 in range(NBLK // GRP):
                ps = psp.tile([128, GRP * 10], F32, tag="ps", name="ps")
                for bb in range(GRP):
                    b = g0 * GRP + bb
                    nc.tensor.matmul(
                        ps[:, bb * 10:(bb + 1) * 10],
                        hp_sb[:, b * 128:b * 128 + 128],
                        tb_sb,
                        start=True,
                        stop=True,
                    )
                dst = ob_sb[:, g0 * GRP * 10:(g0 + 1) * GRP * 10]
                if g0 % 2 == 0:
                    nc.vector.tensor_copy(dst, ps[:])
                else:
                    nc.scalar.activation(
                        dst, ps[:], mybir.ActivationFunctionType.Copy
                    )
            nc.sync.dma_start(ob[:], ob_sb[:])
    nc.finalize()
    return nc


def _chain_host(s1, S, W0, b0, g0, beta0, Ws, bs, gs, betas, Wf, bf):
    """Collapse BN chain on global moments of h = x@W0.T (no bias). float64.
    Returns Tmat [10,10], r [10] with out = h @ Tmat + r."""
    m = s1.astype(np.float64) / B
    C = S.astype(np.float64) / B - np.outer(m, m)
    g0 = g0.astype(np.float64)
    var0 = np.diag(C).copy()
    A = np.diag(g0 / np.sqrt(var0 + EPS))
    d = beta0.astype(np.float64).copy()
    Ws64 = Ws.astype(np.float64)
    gs64 = gs.astype(np.float64)
    betas64 = betas.astype(np.float64)
    for k in range(Ws64.shape[0]):
        Ak = A @ Ws64[k].T
        var = np.einsum("ij,ik,kj->j", Ak, C, Ak)
        A = Ak * (gs64[k] / np.sqrt(var + EPS))[None, :]
        d = betas64[k].copy()
    Tmat = A @ Wf.astype(np.float64).T
    r = d @ Wf.astype(np.float64).T + bf.astype(np.float64)
    # fold bias b0 and centering: out = (h + b0 - (m + b0)) @ Tmat + r
    return Tmat, (r - m @ Tmat)


def kernel(**inputs):
    from concourse.bass_utils import run_bass_kernel_spmd
    import ml_dtypes

    E3 = ml_dtypes.float8_e3m4

    inputs = {k: np.asarray(v, dtype=np.float32) for k, v in inputs.items()}
    x = inputs["x"]
    W0 = inputs["W0"]

    if "nc1" not in _cache:
        _cache["nc1"] = _build_stage1(chunks=STAGE1_CHUNKS)
    if "nc2" not in _cache:
        _cache["nc2"] = _build_stage2()

    # ---- host marshalling for stage 1 ----
    x8 = x.astype(E3)                                    # [B, D] 1 byte/elem
    # w [112, 7, 10] fp16 -> raw bytes as fp8 cols: chunk k = feats k*112..+111
    wb = np.ascontiguousarray(
        W0.T.reshape(KC, KP, 10).transpose(1, 0, 2)
    ).astype(np.float16)                                  # [112, 7, 10]
    wb8 = wb.view(np.uint8).reshape(KP, KC, WCOLS)        # fp16 bytes as uint8

    CHUNKS = STAGE1_CHUNKS
    XTOT = KP * KC * (BC + WCOLS)
    in1 = []
    for c in range(NCORES):
        sl = slice(c * BC, (c + 1) * BC)
        xc = np.ascontiguousarray(
            x8[sl].T.reshape(KC, KP, BC).transpose(1, 0, 2)
        )                                                 # [112, 7, 8192] fp8
        blob = np.empty(XTOT, dtype=np.uint8)
        pos = 0
        off = 0
        for ci, W in enumerate(CHUNKS):
            if ci == 0:
                seg = np.concatenate(
                    [xc[:, :, 0:W].view(np.uint8), wb8], axis=2
                )
            else:
                seg = xc[:, :, off:off + W].view(np.uint8)
            n = seg.size
            blob[pos:pos + n] = seg.ravel()
            pos += n
            off += W
        in1.append({"x8": blob.view(E3)})
    res1 = run_bass_kernel_spmd(_cache["nc1"], in1, core_ids=list(range(NCORES)))

    # ---- gather moments (device blocks 0..55 + host top-up 56..63), chain ----
    s1 = np.zeros(10, dtype=np.float64)
    S = np.zeros((10, 10), dtype=np.float64)
    h_parts = []
    for c in range(NCORES):
        hbc = np.concatenate(
            [np.asarray(res1.results[c]["hba"]), np.asarray(res1.results[c]["hbb"])],
            axis=1,
        )                                                          # [128, HBW] fp16
        mom = np.ascontiguousarray(hbc[0:11, NBLK * 11:NBLK * 11 + 22]).view(
            np.float32
        ).astype(np.float64)                                       # [11, 11]
        s1 += mom[10, 0:10]
        S += mom[0:10, 0:10]
        h16 = hbc[:, 0:NBLK * 11].reshape(128, NBLK, 11)[:, :, 0:10]
        tail = h16[:, MOMBLK:, :].astype(np.float64).reshape(-1, 10)
        S += tail.T @ tail
        s1 += tail.sum(axis=0)
        h_parts.append(h16)

    Tmat, r = _chain_host(
        s1, S,
        W0, inputs["b0"], inputs["g0"], inputs["beta0"],
        inputs["Ws"], inputs["bs"], inputs["gs"], inputs["betas"],
        inputs["Wf"], inputs["bf"],
    )

    # ---- host marshalling for stage 2 ----
    tbv = np.concatenate([Tmat, r[None, :]], axis=0).astype(np.float16)  # [11, 10]
    in2 = []
    for c in range(NCORES):
        ht = h_parts[c].transpose(1, 0, 2).reshape(BC, 10).T       # [10, BC] fp16
        hpc = np.empty((11, BC + 10), dtype=np.float16)
        hpc[0:10, 0:BC] = ht
        hpc[10, 0:BC] = 1.0
        hpc[0:11, BC:BC + 10] = tbv
        in2.append({"hp": hpc})
    res2 = run_bass_kernel_spmd(_cache["nc2"], in2, core_ids=list(range(NCORES)))

    out_parts = []
    for c in range(NCORES):
        obc = np.asarray(res2.results[c]["ob"])           # [128, 640] fp16
        out_parts.append(
            obc.reshape(128, NBLK, 10).transpose(1, 0, 2).reshape(BC, 10)
        )
    return np.ascontiguousarray(
        np.concatenate(out_parts, axis=0).astype(np.float32)
    )
